# revision 1
# baseline (speedup 1.0000x reference)
"""CapsuleLayer dynamic-routing kernel for 8 Trainium2 NeuronCores.

Problem (hardcoded shapes):
  x [512, 1152, 8] f32, W [10, 1152, 8, 16] f32
  priors = einsum('bri,nrio->nbro'); 3 rounds of softmax-over-R routing.
  out [10, 512, 1, 1, 16] f32.

Sharding: 4-way batch (128 each) x 2-way capsule N (5 each) over 8 cores.
Per core / per capsule n:
  1. PE builds priors P[b, r, o] with block-diagonal-W matmuls
     (k = 16 r's x 8 i's = 128, lhsT = x-transposed chunk, rhs = blockdiag W).
  2. ACT copies PSUM waves into SBUF as P_A [b, (o, r)] bf16; round-1's
     uniform sum rides along on the PE as a second accumulating matmul that
     reuses the stationary x chunk (fp32, frees the DVE reduce).
  3. Rounds: weighted sums as batched bf16 products + 3D tensor_reduce over
     innermost r (fp32 accumulation); logit updates via 4 bf16 sub-chains
     (2 DVE stt-chains, 2 GPSIMD TT-chains with stride-0 scalar broadcast)
     combined in fp32 mostly on GPSIMD; exp on ACT with fused Z accumulation;
     logits kept fp32.

The capsule loop is software-pipelined: for step k the program emits
  create(k) -> squash1(k) -> GPS half of LU1(k) -> full rounds tail(k-1)
  -> DVE half of LU1(k) + final L1(k) combine
so GPSIMD runs capsule k's first logit update while DVE runs capsule k-1's
weighted sums. Engine splits are coarse-grained on purpose — per-element
cross-engine ping-pong measured ~2x slower.
"""

import numpy as np

B, R, I, O, N = 512, 1152, 8, 16, 10
BG, NG = 4, 2              # batch groups x capsule groups = 8 cores
BL, NL = B // BG, N // NG  # 128, 5
RB = R // 16               # 72 r-blocks of 16
WV = 6                     # r-blocks per PSUM wave
QRB = 18                   # r-blocks per Wbd quarter buffer
NQ = RB // QRB             # 4
NCORES = 8

_CACHE = {}


def _build_program(debug=False, repeat=1):
    import concourse.tile as tile
    from concourse import bacc, mybir

    F32 = mybir.dt.float32
    BF16 = mybir.dt.bfloat16
    ALU = mybir.AluOpType
    ACTF = mybir.ActivationFunctionType
    AX = mybir.AxisListType

    nc = bacc.Bacc("TRN2", target_bir_lowering=False, debug=debug,
                   num_devices=NCORES)

    xT_d = nc.dram_tensor("xT", [128, RB * BL], F32, kind="ExternalInput")
    Wsl_d = nc.dram_tensor("Wsl", [NL, 16, 8, RB, O], F32, kind="ExternalInput")
    out_d = nc.dram_tensor("out", [128, NL * O], F32, kind="ExternalOutput")

    with tile.TileContext(nc) as tc:
        with (
            tc.tile_pool(name="const", bufs=1) as cpool,
            tc.tile_pool(name="psum", bufs=2, space="PSUM") as pspool,
            tc.tile_pool(name="big", bufs=1) as bigpool,
            tc.tile_pool(name="small", bufs=4) as smpool,
        ):
            xT = cpool.tile([128, RB * BL], F32, tag="xT")
            outacc = cpool.tile([128, NL * O], F32, tag="outacc")
            # two persistent block-diag W buffers (quarters), zeroed once;
            # off-diagonal slots stay zero forever, diag slots re-DMAed per use
            wbd = [cpool.tile([128, QRB, 16 * O], F32, name=f"wbd{j}")
                   for j in range(2)]
            # dense W [p=(d,i), rb, o] per capsule — feeds the R1 sum matmuls
            wd2 = [cpool.tile([128, RB, O], F32, name=f"wd2_{j}")
                   for j in range(2)]
            # priors, o-outer layout [b, o, r]; 2 buffers (create k || tail k-1)
            pa = [cpool.tile([128, O, R], BF16, name=f"pa{j}") for j in range(2)]
            # WS product scratch; row 0 doubles as the DVE chain scratch
            wjunk = cpool.tile([128, 4, R], BF16, tag="wjunk")
            gscratch = cpool.tile([128, R], BF16, tag="gscratch")
            gscr = [cpool.tile([128, R], BF16, name=f"gscr{j}") for j in range(2)]
            cq = [cpool.tile([128, R], BF16, name=f"cq{j}") for j in range(4)]
            # rotated logits-after-round-1
            l1t = [cpool.tile([128, R], F32, name=f"l1t{j}") for j in range(2)]

            nc.sync.dma_start(xT[:], xT_d[:])
            nc.gpsimd.memset(wbd[0][:], 0.0)
            nc.gpsimd.memset(wbd[1][:], 0.0)

            def squash(su, z_recip_ap, dst_v):
                # v = s * sqrt(n2) / (1 + n2),  s = su / Z,  n2 = sum(s^2)
                s = smpool.tile([128, O], F32, tag="s")
                if z_recip_ap is None:
                    nc.vector.tensor_scalar_mul(s[:], su[:], 1.0 / R)
                else:
                    nc.vector.tensor_scalar_mul(s[:], su[:], z_recip_ap)
                sqj = smpool.tile([128, O], F32, tag="sqj")
                n2 = smpool.tile([128, 1], F32, tag="n2")
                nc.scalar.activation(sqj[:], s[:], ACTF.Square, accum_out=n2[:])
                rt = smpool.tile([128, 1], F32, tag="rt")
                nc.scalar.activation(rt[:], n2[:], ACTF.Sqrt)
                u = smpool.tile([128, 1], F32, tag="u")
                nc.vector.tensor_scalar_add(u[:], n2[:], 1.0)
                rr = smpool.tile([128, 1], F32, tag="rr")
                nc.vector.reciprocal(rr[:], u[:])
                sc = smpool.tile([128, 1], F32, tag="sc")
                nc.vector.tensor_mul(sc[:], rt[:], rr[:])
                nc.vector.tensor_scalar_mul(dst_v, s[:], sc[:])

            def lu_chains_dve(pan, v, qs):
                s_ = wjunk[:, 0, :]
                for q in qs:
                    base = 4 * q
                    nc.vector.tensor_scalar_mul(s_, pan[:, base, :],
                                                v[:, base:base + 1])
                    hop = [s_, cq[q][:], s_, cq[q][:]]
                    for j in (1, 2, 3):
                        nc.vector.scalar_tensor_tensor(
                            hop[j], pan[:, base + j, :],
                            v[:, base + j:base + j + 1], hop[j - 1],
                            op0=ALU.mult, op1=ALU.add)

            def lu_chains_gps(pan, v, qs):
                # gpsimd has no TensorScalarPtr on TRN2; TT pairs with a
                # stride-0 broadcast scalar, 3-buffer rotation
                for q in qs:
                    bufs3 = [gscratch, gscr[0], gscr[1]]
                    base = 4 * q
                    nc.gpsimd.tensor_tensor(
                        bufs3[0][:], pan[:, base, :],
                        v[:, base:base + 1].broadcast_to([128, R]),
                        op=ALU.mult)
                    prev = bufs3[0]
                    for j in (1, 2, 3):
                        pr = bufs3[(2 * j - 1) % 3]
                        nc.gpsimd.tensor_tensor(
                            pr[:], pan[:, base + j, :],
                            v[:, base + j:base + j + 1].broadcast_to([128, R]),
                            op=ALU.mult)
                        dst = cq[q] if j == 3 else bufs3[(2 * j) % 3]
                        nc.gpsimd.tensor_tensor(dst[:], prev[:], pr[:],
                                                op=ALU.add)
                        prev = dst

            def logit_update(pan, v, l_prev, l_dst):
                # full LU (used in the tail stage for round 2)
                lu_chains_dve(pan, v, (0, 1))
                lu_chains_gps(pan, v, (2, 3))
                t01 = bigpool.tile([128, R], F32, tag="t01")
                t23 = bigpool.tile([128, R], F32, tag="t23")
                nc.vector.tensor_add(t01[:], cq[0][:], cq[1][:])
                nc.gpsimd.tensor_tensor(t23[:], cq[2][:], cq[3][:], op=ALU.add)
                nc.gpsimd.tensor_tensor(l_dst[:], l_prev[:], t01[:], op=ALU.add)
                nc.gpsimd.tensor_tensor(l_dst[:], l_dst[:], t23[:], op=ALU.add)

            def weighted_sum(pan, e_bf, su):
                # batched: bf16 product of 2 o-rows at a time (e broadcast
                # over o via a stride-0 dim), then one 3D reduce over
                # innermost r with fp32 accumulation
                for h in range(4):
                    os_ = slice(h * 4, (h + 1) * 4)
                    e3d = e_bf[:].unsqueeze(1).broadcast_to([128, 4, R])
                    nc.vector.tensor_mul(wjunk[:], pan[:, os_, :], e3d)
                    nc.vector.tensor_reduce(su[:, os_], wjunk[:],
                                            axis=AX.X, op=ALU.add)

            def create_stage(nr):
                n = nr % NL
                pan = pa[nr % 2]
                wd = wd2[nr % 2]
                nc.sync.dma_start(
                    wd[:], Wsl_d[n].rearrange("d i rb o -> (d i) rb o"))
                su_ps = pspool.tile([128, O], F32, tag="su_ps")
                for q in range(NQ):
                    wq = wbd[(nr * NQ + q) % 2]
                    for d in range(16):
                        nc.sync.dma_start(
                            wq[d * 8:(d + 1) * 8, :, d * O:(d + 1) * O],
                            Wsl_d[n, d, :, q * QRB:(q + 1) * QRB, :])
                    for wl in range(QRB // WV):
                        wave = pspool.tile([128, WV * 16 * O], F32, tag="wave")
                        for k in range(WV):
                            rb = q * QRB + wl * WV + k
                            nc.tensor.matmul(
                                wave[:, k * 256:(k + 1) * 256],
                                xT[:, rb * BL:(rb + 1) * BL],
                                wq[:, wl * WV + k, :],
                                start=True, stop=True)
                            # R1 uniform sum: shares the stationary x chunk
                            nc.tensor.matmul(
                                su_ps[:], xT[:, rb * BL:(rb + 1) * BL],
                                wd[:, rb, :],
                                start=(rb == 0), stop=(rb == RB - 1),
                                skip_group_check=True)
                        # PSUM wave [p, (rb6, r2_16, o16)] -> pa [p, o, r] slice
                        w0 = (q * QRB + wl * WV) * 16
                        dst = pan[:, :, w0:w0 + WV * 16].rearrange(
                            "p o (rb r2) -> p o rb r2", rb=WV)
                        src = wave.rearrange(
                            "p (rb r2 o) -> p o rb r2", rb=WV, r2=16, o=O)
                        nc.scalar.copy(dst, src)
                return su_ps

            def lu1_all_gps(nr, su_ps):
                # squash of the uniform round + ALL of LU1(k) on GPSIMD:
                # l1(k) completes while DVE executes the k-1 tail, keeping
                # the DVE queue free of step-boundary work
                pan = pa[nr % 2]
                v1 = smpool.tile([128, O], F32, tag="v1")
                squash(su_ps, None, v1[:])
                # chains accumulate straight into l1 via GPS-private
                # buffers ONLY (gscratch/gscr/l1t). Any tile also written by
                # the k-1 tail (cq, t01/t23) would stall that tail's engine
                # queue behind all of LU1(k)'s GPS ops — measured to erase
                # the entire pipelining gain.
                l1 = l1t[nr % 2]
                bufs3 = [gscratch, gscr[0], gscr[1]]
                for q in range(4):
                    base = 4 * q
                    nc.gpsimd.tensor_tensor(
                        bufs3[0][:], pan[:, base, :],
                        v1[:, base:base + 1].broadcast_to([128, R]),
                        op=ALU.mult)
                    prev = bufs3[0]
                    for j in (1, 2, 3):
                        pr = bufs3[(2 * j - 1) % 3]
                        nc.gpsimd.tensor_tensor(
                            pr[:], pan[:, base + j, :],
                            v1[:, base + j:base + j + 1].broadcast_to([128, R]),
                            op=ALU.mult)
                        # chain 0's final add lands straight in fp32 l1
                        # (bf16+bf16->f32 Pool add, the proven pattern);
                        # walrus rejects max/copy on Pool
                        dst = l1 if (q == 0 and j == 3) else bufs3[(2 * j) % 3]
                        nc.gpsimd.tensor_tensor(dst[:], prev[:], pr[:],
                                                op=ALU.add)
                        prev = dst
                    if q > 0:
                        nc.gpsimd.tensor_tensor(l1[:], l1[:], prev[:],
                                                op=ALU.add)
                return l1

            def tail_stage(nr, l1):
                n = nr % NL
                pan = pa[nr % 2]
                e2 = bigpool.tile([128, R], BF16, tag="e")
                z2 = smpool.tile([128, 1], F32, tag="z2")
                nc.scalar.activation(e2[:], l1[:], ACTF.Exp, accum_out=z2[:])
                rz2 = smpool.tile([128, 1], F32, tag="rz2")
                nc.vector.reciprocal(rz2[:], z2[:])
                su2 = smpool.tile([128, O], F32, tag="su2")
                weighted_sum(pan, e2, su2)
                v2 = smpool.tile([128, O], F32, tag="v2")
                squash(su2, rz2[:], v2[:])

                l2 = bigpool.tile([128, R], F32, tag="l2")
                logit_update(pan, v2, l1, l2)

                e3 = bigpool.tile([128, R], BF16, tag="e")
                z3 = smpool.tile([128, 1], F32, tag="z3")
                nc.scalar.activation(e3[:], l2[:], ACTF.Exp, accum_out=z3[:])
                rz3 = smpool.tile([128, 1], F32, tag="rz3")
                nc.vector.reciprocal(rz3[:], z3[:])
                su3 = smpool.tile([128, O], F32, tag="su3")
                weighted_sum(pan, e3, su3)
                squash(su3, rz3[:], outacc[:, n * O:(n + 1) * O])

            # software-pipelined capsule loop
            NT = repeat * NL
            prev = None  # (nr, l1)
            for nr in range(NT):
                su_ps = create_stage(nr)
                l1 = lu1_all_gps(nr, su_ps)
                if prev is not None:
                    tail_stage(*prev)
                prev = (nr, l1)
            tail_stage(*prev)

            nc.sync.dma_start(out_d[:], outacc[:])

    nc.compile()
    return nc


def _host_prep(x, W):
    """Build per-core input maps."""
    x = np.ascontiguousarray(x, dtype=np.float32)
    W = np.ascontiguousarray(W, dtype=np.float32)
    in_maps = []
    for c in range(NCORES):
        bg, ng = c % BG, c // BG
        xs = x[bg * BL:(bg + 1) * BL]                      # [128, 1152, 8]
        # xT[p=(r16*8+i), rb*128+b] = xs[b, rb*16+r16, i]
        xT = np.ascontiguousarray(
            xs.reshape(BL, RB, 16, 8).transpose(2, 3, 1, 0).reshape(128, RB * BL))
        Wn = W[ng * NL:(ng + 1) * NL].reshape(NL, RB, 16, 8, O)
        # Wsl[n, d, i, rb, o] = W[n, rb*16+d, i, o]
        Wsl = np.ascontiguousarray(Wn.transpose(0, 2, 3, 1, 4))
        in_maps.append({"xT": xT, "Wsl": Wsl})
    return in_maps


def _gather(results):
    out = np.zeros((N, B, 1, 1, O), np.float32)
    for c in range(NCORES):
        bg, ng = c % BG, c // BG
        o = results[c]["out"].reshape(BL, NL, O)           # [b, n, o]
        out[ng * NL:(ng + 1) * NL, bg * BL:(bg + 1) * BL, 0, 0, :] = \
            o.transpose(1, 0, 2)
    return out


def kernel(x, W):
    from concourse.bass_utils import run_bass_kernel_spmd
    if "nc" not in _CACHE:
        _CACHE["nc"] = _build_program()
    nc = _CACHE["nc"]
    in_maps = _host_prep(x, W)
    res = run_bass_kernel_spmd(nc, in_maps, core_ids=list(range(NCORES)))
    _CACHE["last_results"] = res
    return _gather(res.results)


if __name__ == "__main__":
    d = np.load("/root/problem/work/ref.npz")
    out = kernel(d["x"], d["W"])
    exp = d["expected"]
    rel = np.linalg.norm(out - exp) / np.linalg.norm(exp)
    print("rel err:", rel)

